# revision 42
# baseline (speedup 1.0000x reference)
"""Causal self-attention (B=2, T=2048, C=1024, H=16) on 8 trn2 NeuronCores.

Collective-free, single uniform SPMD program, mod-4 query-row interleaving.

Sharding: core = (batch b = core//4, slot c = core%4). Slot c owns query
rows {r : r % 4 == c} (512 rows). Gathered q-tile j holds global rows
4*(128j+i)+c, so causal coverage is k-tiles 0..4j+3 for every slot —
the program structure is identical on all cores; only the diagonal mask
data (shift by c) and the gathered xq columns differ per core.

V5: all-bf16 dataflow (x, W, K^T, Q^T — halves DMA and SBUF traffic vs
f32r), tt-major x streaming so the V pass starts as soon as the first
128-row token tile of x lands instead of after the full 8MB load.

Per head, attention runs per k-tile kt with the active query suffix
cols [128*(kt//4) : 512]; exp is batched over 2 k-tiles; only the
newest (diagonal) q-tile needs a mask multiply. Issue order is
software-pipelined (AV of group g-1 issued after S_T/exp of group g) so
the tensor engine never idles behind the ACT->DVE chain.

K/V computed for all 2048 rows per core (the redundancy that buys zero
communication), transposed dataflow with the softmax denominator from
a ones-column in V.
"""

import sys

for _p in ("/opt/trn_rl_repo", "/root/.axon_site/_ro/trn_rl_repo"):
    if _p not in sys.path:
        sys.path.append(_p)

import numpy as np

import concourse.bass as bass
import concourse.mybir as mybir
import concourse.tile as tile
from concourse import bacc

DIM = 1024
N_HEADS = 16
B = 2
T = 2048
KT = DIM // 128
TT = T // 128
SCALE = 1.0 / 8.0
# W_k is pre-scaled by 32 on the host so its fp8e4m3 encodings stay in
# the normal range (raw values ~uniform(-1/32,1/32) are half-subnormal);
# scores come out 32x large, compensated in the exp scale.
WQK_PRESCALE = 32.0
SCORE_SCALE = SCALE / WQK_PRESCALE
N_CORES = 8
ROWS_PER_CORE = 512

F32R = mybir.dt.float32r
BF16 = mybir.dt.bfloat16
F32 = mybir.dt.float32

_CACHE = {}


def build_nc(phases="all", reps=1, probe=None):
    # probe: None (normal) | "noexp" (DVE copy instead of ACT exp — wrong
    # numerics, timing experiments only)
    nc = bacc.Bacc(None)

    F8 = mybir.dt.float8e4
    xT = nc.dram_tensor("xT", [128, KT * T], BF16, kind="ExternalInput")
    # fp8 copy with kt-pair interleave for the DoubleRow K projection;
    # Q stays bf16 (fp8 on both q and k puts rel_err too close to the gate)
    xp8 = nc.dram_tensor("xp8", [128, KT * T], F8, kind="ExternalInput")
    xq = nc.dram_tensor("xq", [128, KT * 512], BF16, kind="ExternalInput")
    wq = nc.dram_tensor("wq", [128, KT * 1024], BF16, kind="ExternalInput")
    wk = nc.dram_tensor("wk", [128, KT * 1024], F8, kind="ExternalInput")
    wv = nc.dram_tensor("wv", [128, KT * 1024], BF16, kind="ExternalInput")
    wo = nc.dram_tensor("wo", [128, KT * 1024], BF16, kind="ExternalInput")
    qmask = nc.dram_tensor("qmask", [128, TT * 128], BF16, kind="ExternalInput")
    ones_bf = nc.dram_tensor("ones_bf", [128, 64], BF16, kind="ExternalInput")
    ones_fr = nc.dram_tensor("ones_fr", [128, 64], F32R, kind="ExternalInput")
    out = nc.dram_tensor("out", [ROWS_PER_CORE, DIM], F32, kind="ExternalOutput")

    n_pairs = {"v": 0, "vk": 8, "vka": 8, "all": 8}[phases]
    do_v = True
    do_attn = phases in ("vka", "all")
    do_proj = phases == "all"

    with tile.TileContext(nc) as tc:
        with tc.tile_pool(name="persist", bufs=1) as pers, \
             tc.tile_pool(name="work", bufs=3) as work, \
             tc.tile_pool(name="epool", bufs=11) as epool, \
             tc.tile_pool(name="wstream", bufs=2) as wstream, \
             tc.tile_pool(name="pavsb", bufs=8) as pavsb, \
             tc.tile_pool(name="psmm", bufs=2, space="PSUM") as psmm, \
             tc.tile_pool(name="psst", bufs=2, space="PSUM") as psst, \
             tc.tile_pool(name="psav", bufs=2, space="PSUM") as psav:

            ones_sb = pers.tile([128, 64], F32R, tag="ones")
            onesb_sb = pers.tile([128, 64], BF16, tag="onesb")
            qm_sb = pers.tile([128, TT * 128], BF16, tag="qmask")
            xT4 = xT[:].rearrange("p (k t c) -> p k t c", k=KT, t=TT, c=128)
            xp8_sb = pers.tile([128, KT * T], F8, tag="xp8")
            # [p, jpair, e, t] — e selects kt=2j+e (DoubleRow pair)
            xp3 = xp8_sb[:].rearrange("p (j e t) -> p j e t", j=KT // 2, e=2)
            xq_sb = pers.tile([128, KT * 512], BF16, tag="xq")
            v_sb = pers.tile([128, TT * 1040], BF16, tag="v")
            v4 = v_sb[:].rearrange("p (t h e) -> p t h e", t=TT, h=N_HEADS, e=65)
            aot_sb = [
                pers.tile([128, 512], BF16, tag=f"aot{p}", name=f"aot{p}")
                for p in range(8)
            ]
            wo_sb = pers.tile([128, KT * DIM], BF16, tag="wo")

            for _rep in range(reps):
                nc.sync.dma_start(out=ones_sb[:], in_=ones_fr[:])
                nc.sync.dma_start(out=onesb_sb[:], in_=ones_bf[:])
                nc.sync.dma_start(out=qm_sb[:], in_=qmask[:])
                for h in range(N_HEADS):
                    nc.vector.tensor_copy(
                        out=v4[:, :, h, 64:65],
                        in_=onesb_sb[:, 0:TT].rearrange("p (t o) -> p t o", o=1),
                    )
                if not do_attn:
                    for p_ in range(8):
                        nc.vector.tensor_copy(out=aot_sb[p_][:, 0:64], in_=onesb_sb[:])

                # ---- V pass, pipelined with tt-major x streaming ----
                # x (bf16) is only read by the V pass; scope it so its SBUF
                # is reclaimed for the attention working set
                with tc.tile_pool(name=f"wvpool{_rep}", bufs=1) as wvpool:
                    x_sb = wvpool.tile(
                        [128, KT * T], BF16, tag="x", name=f"x_{_rep}"
                    )
                    x4 = x_sb[:].rearrange(
                        "p (k t c) -> p k t c", k=KT, t=TT, c=128
                    )
                    wv_sb = wvpool.tile(
                        [128, KT * 1024], BF16, tag="wv", name=f"wv_{_rep}"
                    )
                    # order: wv half0, first x tiles, wv half1, rest of x —
                    # V math starts after ~1.25MB of DMA instead of ~2.25MB
                    nc.sync.dma_start(out=wv_sb[:, 0:4096], in_=wv[:, 0:4096])
                    # x arrives token-tile-major: all 8 kt slices of tile tt
                    # in one strided DMA, so V math starts after tile 0
                    for tt in range(2):
                        nc.sync.dma_start(out=x4[:, :, tt, :], in_=xT4[:, :, tt, :])
                    nc.sync.dma_start(
                        out=wv_sb[:, 4096:8192], in_=wv[:, 4096:8192]
                    )
                    for tt in range(2, TT):
                        nc.sync.dma_start(out=x4[:, :, tt, :], in_=xT4[:, :, tt, :])
                    nc.sync.dma_start(out=xp8_sb[:], in_=xp8[:])
                    nc.sync.dma_start(out=xq_sb[:], in_=xq[:])
                    if do_proj:
                        # prefetch: proj weights land during attention
                        for q2 in range(2):
                            nc.sync.dma_start(
                                out=wo_sb[:, q2 * 4096 : (q2 + 1) * 4096],
                                in_=wo[:, q2 * 4096 : (q2 + 1) * 4096],
                            )
                    if do_v:
                        for tt in range(TT):
                            for half in range(2):
                                vacc = psmm.tile([128, 512], F32, tag="mm512")
                                for kt in range(KT):
                                    nc.tensor.matmul(
                                        vacc[:],
                                        x4[:, kt, tt, :],
                                        wv_sb[:, half * 4096 + kt * 512 : half * 4096 + kt * 512 + 512],
                                        start=(kt == 0), stop=(kt == KT - 1),
                                    )
                                nc.vector.tensor_copy(
                                    out=v4[:, tt, half * 8 : half * 8 + 8, 0:64],
                                    in_=vacc[:].rearrange("p (h e) -> p h e", h=8, e=64),
                                )

                # ---- per head-pair: K.T, Q.T (fp8 DoubleRow), attention ----
                pav_drained = []
                for p in range(n_pairs):
                    wk_p = wstream.tile([128, KT * 128], F8, tag="wkp")
                    wq_p = wstream.tile([128, KT * 128], BF16, tag="wqp")
                    nc.sync.dma_start(
                        out=wk_p[:], in_=wk[:, p * 1024 : p * 1024 + 1024]
                    )
                    nc.sync.dma_start(
                        out=wq_p[:], in_=wq[:, p * 1024 : p * 1024 + 1024]
                    )
                    wk3 = wk_p[:].rearrange("p (j e d) -> p j e d", j=KT // 2, e=2)

                    ktp = wstream.tile([128, T], BF16, tag="ktp")
                    for tch in range(4):
                        kacc = psmm.tile([128, 512], F32, tag="mm512")
                        for j in range(KT // 2):
                            nc.tensor.matmul(
                                kacc[:],
                                wk3[:, j],
                                xp3[:, j, :, tch * 512 : tch * 512 + 512],
                                start=(j == 0), stop=(j == KT // 2 - 1),
                                perf_mode=mybir.MatmulPerfMode.DoubleRow,
                            )
                        nc.vector.tensor_copy(
                            out=ktp[:, tch * 512 : tch * 512 + 512], in_=kacc[:]
                        )

                    qtp = wstream.tile([128, 512], BF16, tag="qtp")
                    qacc = psmm.tile([128, 512], F32, tag="mm512")
                    for kt in range(KT):
                        nc.tensor.matmul(
                            qacc[:],
                            wq_p[:, kt * 128 : kt * 128 + 128],
                            xq_sb[:, kt * 512 : kt * 512 + 512],
                            start=(kt == 0), stop=(kt == KT - 1),
                        )
                    nc.vector.tensor_copy(out=qtp[:], in_=qacc[:])

                    if do_attn:
                        pavs = [psav.tile([65, 512], F32, tag="pav", name=f"pav{p}_{half_}_{_rep}") for half_ in range(2)]
                        pending = []  # AV issue lag-2 behind S/exp
                        for g in range(8):
                            j = g // 2
                            qs = 128 * j
                            N = 512 - qs
                            e2s = []
                            for half in range(2):
                                lo, hi = half * 64, half * 64 + 64
                                e2 = epool.tile([128, 1024], BF16, tag="e2")
                                e3 = e2[:].rearrange("p (s m) -> p s m", s=2)
                                st2 = psst.tile([128, 1024], F32, tag="st")
                                st3 = st2[:].rearrange("p (s m) -> p s m", s=2)
                                for s in range(2):
                                    kt = 2 * g + s
                                    nc.tensor.matmul(
                                        st2[:, s * 512 : s * 512 + N],
                                        ktp[lo:hi, kt * 128 : kt * 128 + 128],
                                        qtp[lo:hi, qs:512],
                                        start=True, stop=True,
                                    )
                                if probe == "noexp":
                                    nc.vector.tensor_copy(
                                        out=e3[:, :, 0:N], in_=st3[:, :, 0:N]
                                    )
                                else:
                                    nc.scalar.activation(
                                        e3[:, :, 0:N], st3[:, :, 0:N],
                                        mybir.ActivationFunctionType.Exp,
                                        scale=SCORE_SCALE,
                                    )
                                nc.vector.tensor_mul(
                                    out=e3[:, :, 0:128],
                                    in0=e3[:, :, 0:128],
                                    in1=qm_sb[:, 2 * g * 128 : 2 * g * 128 + 256].rearrange(
                                        "p (s m) -> p s m", s=2
                                    ),
                                )
                                e2s.append(e2)
                            pending.append((e2s, j, N, g))
                            if len(pending) > 4:
                                _issue_av2(nc, pavs, v_sb, pending.pop(0), p)
                        for pend in pending:
                            _issue_av2(nc, pavs, v_sb, pend, p)

                        # drain pav to SBUF; normalization happens after the
                        # pair loop so its recip->matmul chains never stall
                        # the hot PE stream
                        pv = pavsb.tile([65, 1024], BF16, tag="pavS",
                                        name=f"pavS{p}_{_rep}")
                        pav_drained.append(pv)
                        for half in range(2):
                            nc.vector.tensor_copy(
                                out=pv[:, half * 512 : half * 512 + 512],
                                in_=pavs[half][:],
                            )

                # ---- deferred softmax normalization ----
                if do_attn:
                    for p in range(n_pairs):
                        pv = pav_drained[p]
                        for half in range(2):
                            lo, hi = half * 64, half * 64 + 64
                            sl = pv[:, half * 512 : half * 512 + 512]
                            recip = work.tile([128, 512], F32R, tag="recip")
                            with nc.allow_low_precision(reason="softmax recip; f32r ~1e-4"):
                                nc.vector.reciprocal(out=recip[64:65, :], in_=sl[64:65, :])
                            pb = psmm.tile([64, 512], F32, tag="mm512")
                            nc.tensor.matmul(
                                pb[:], ones_sb[64:65, 0:64], recip[64:65, :],
                                start=True, stop=True,
                            )
                            nc.vector.tensor_mul(
                                out=aot_sb[p][lo:hi, :], in0=sl[0:64, :], in1=pb[:]
                            )

                # ---- projection ----
                if do_proj:
                    for qi in range(4):
                        for fc in range(2):
                            yp = psmm.tile([128, 512], F32, tag="mm512")
                            for p in range(8):
                                nc.tensor.matmul(
                                    yp[:],
                                    aot_sb[p][:, qi * 128 : qi * 128 + 128],
                                    wo_sb[:, p * DIM + fc * 512 : p * DIM + fc * 512 + 512],
                                    start=(p == 0), stop=(p == 7),
                                )
                            y_sb = work.tile([128, 512], F32, tag="y")
                            nc.vector.tensor_copy(out=y_sb[:], in_=yp[:])
                            nc.sync.dma_start(
                                out=out[qi * 128 : qi * 128 + 128, fc * 512 : fc * 512 + 512],
                                in_=y_sb[:],
                            )

    nc.finalize()
    return nc


def _issue_av2(nc, pavs, v_sb, pend, p):
    e2s, j, N, g = pend
    qs = 128 * j
    for half in range(2):
        h = 2 * p + half
        for s in range(2):
            kt = 2 * g + s
            nc.tensor.matmul(
                pavs[half][:, qs:512],
                v_sb[:, kt * 1040 + h * 65 : kt * 1040 + h * 65 + 65],
                e2s[half][:, s * 512 : s * 512 + N],
                start=(kt == 0), stop=(kt == TT - 1),
            )


def make_in_maps(x, W_qkv, W_proj):
    import ml_dtypes

    bf = ml_dtypes.bfloat16
    f8 = mybir.dt.np(mybir.dt.float8e4)
    x = np.asarray(x, dtype=np.float32)
    W_qkv = np.asarray(W_qkv, dtype=np.float32)
    W_proj = np.asarray(W_proj, dtype=np.float32)
    W_q, W_k, W_v = W_qkv[:DIM], W_qkv[DIM:2 * DIM], W_qkv[2 * DIM:]

    def pair_major_f8(W):
        # [p, head-pair, jpair, e, d] with kt = 2j+e (DoubleRow interleave);
        # prescaled so fp8 encodings avoid the subnormal range
        WT = (W.T * WQK_PRESCALE).reshape(KT // 2, 2, 128, 8, 128)
        return np.ascontiguousarray(
            WT.transpose(2, 3, 0, 1, 4).reshape(128, KT * 1024)
        ).astype(f8)

    def pair_major(W):
        WT = W.T.reshape(KT, 128, 8, 128)
        return np.ascontiguousarray(
            WT.transpose(1, 2, 0, 3).reshape(128, KT * 1024)
        ).astype(bf)

    wq_d = pair_major(W_q)
    wk_d = pair_major_f8(W_k)
    wv_d = np.ascontiguousarray(
        W_v.T.reshape(KT, 128, 2, 512).transpose(1, 2, 0, 3).reshape(128, KT * 1024)
    ).astype(bf)
    wo_d = np.ascontiguousarray(
        W_proj.T.reshape(KT, 128, DIM).transpose(1, 0, 2).reshape(128, KT * DIM)
    ).astype(bf)

    in_maps = []
    for core in range(N_CORES):
        b, c = core // 4, core % 4
        xb = x[b]
        xTl = np.ascontiguousarray(
            xb.T.reshape(KT, 128, T).transpose(1, 0, 2).reshape(128, KT * T)
        ).astype(bf)
        # fp8 kt-pair-interleaved copy for the DoubleRow K projection
        xp8l = np.ascontiguousarray(
            xb.T.reshape(KT // 2, 2, 128, T).transpose(2, 0, 1, 3).reshape(128, KT * T)
        ).astype(f8)
        qrows = np.arange(512) * 4 + c  # gathered rows, ascending
        xql = np.ascontiguousarray(
            xb[qrows].T.reshape(KT, 128, 512).transpose(1, 0, 2).reshape(128, KT * 512)
        ).astype(bf)
        # diag masks: for k-tile kt, q-tile j=kt//4 occupies active cols [0:128]:
        # qmask[:, kt*128 + i] = 1 if kt*128 + p <= qrows[128*(kt//4) + i]
        p_idx = np.arange(128)[:, None]
        qm = np.empty((128, TT * 128), dtype=bf)
        for kt in range(TT):
            jj = kt // 4
            qr = qrows[128 * jj : 128 * jj + 128][None, :]
            qm[:, kt * 128 : kt * 128 + 128] = (kt * 128 + p_idx <= qr).astype(bf)
        in_maps.append({
            "xT": xTl, "xp8": xp8l, "xq": xql,
            "wq": wq_d, "wk": wk_d, "wv": wv_d, "wo": wo_d,
            "qmask": qm,
            "ones_bf": np.ones((128, 64), bf),
            "ones_fr": np.ones((128, 64), np.float32),
        })
    return in_maps


def assemble_output(results):
    y = np.empty((B, T, DIM), dtype=np.float32)
    for core in range(N_CORES):
        b, c = core // 4, core % 4
        y[b, np.arange(512) * 4 + c] = results[core]["out"]
    return y


def kernel(x, W_qkv, W_proj):
    from concourse.bass_utils import run_bass_kernel_spmd

    if "nc" not in _CACHE:
        _CACHE["nc"] = build_nc()
    nc = _CACHE["nc"]
    in_maps = make_in_maps(x, W_qkv, W_proj)
    res = run_bass_kernel_spmd(nc, in_maps, list(range(N_CORES)))
    return assemble_output(res.results)


# revision 45
# speedup vs baseline: 1.0750x; 1.0750x over previous
"""Causal self-attention (B=2, T=2048, C=1024, H=16) on 8 trn2 NeuronCores.

Collective-free, single uniform SPMD program, mod-4 query-row interleaving.

Sharding: core = (batch b = core//4, slot c = core%4). Slot c owns query
rows {r : r % 4 == c} (512 rows). Gathered q-tile j holds global rows
4*(128j+i)+c, so causal coverage is k-tiles 0..4j+3 for every slot —
the program structure is identical on all cores; only the diagonal mask
data (shift by c) and the gathered xq columns differ per core.

V5: all-bf16 dataflow (x, W, K^T, Q^T — halves DMA and SBUF traffic vs
f32r), tt-major x streaming so the V pass starts as soon as the first
128-row token tile of x lands instead of after the full 8MB load.

Per head, attention runs per k-tile kt with the active query suffix
cols [128*(kt//4) : 512]; exp is batched over 2 k-tiles; only the
newest (diagonal) q-tile needs a mask multiply. Issue order is
software-pipelined (AV of group g-1 issued after S_T/exp of group g) so
the tensor engine never idles behind the ACT->DVE chain.

K/V computed for all 2048 rows per core (the redundancy that buys zero
communication), transposed dataflow with the softmax denominator from
a ones-column in V.
"""

import sys

for _p in ("/opt/trn_rl_repo", "/root/.axon_site/_ro/trn_rl_repo"):
    if _p not in sys.path:
        sys.path.append(_p)

import numpy as np

import concourse.bass as bass
import concourse.mybir as mybir
import concourse.tile as tile
from concourse import bacc

DIM = 1024
N_HEADS = 16
B = 2
T = 2048
KT = DIM // 128
TT = T // 128
SCALE = 1.0 / 8.0
# W_k is pre-scaled by 32 on the host so its fp8e4m3 encodings stay in
# the normal range (raw values ~uniform(-1/32,1/32) are half-subnormal);
# scores come out 32x large, compensated in the exp scale.
WQK_PRESCALE = 32.0
SCORE_SCALE = SCALE / WQK_PRESCALE
N_CORES = 8
ROWS_PER_CORE = 512

F32R = mybir.dt.float32r
BF16 = mybir.dt.bfloat16
F32 = mybir.dt.float32

_CACHE = {}


def build_nc(phases="all", reps=1, probe=None):
    # probe: None (normal) | "noexp" (DVE copy instead of ACT exp — wrong
    # numerics, timing experiments only)
    nc = bacc.Bacc(None)

    F8 = mybir.dt.float8e4
    xT = nc.dram_tensor("xT", [128, KT * T], BF16, kind="ExternalInput")
    # fp8 copy with kt-pair interleave for the DoubleRow K projection;
    # Q stays bf16 (fp8 on both q and k puts rel_err too close to the gate)
    xp8 = nc.dram_tensor("xp8", [128, KT * T], F8, kind="ExternalInput")
    xq = nc.dram_tensor("xq", [128, KT * 512], BF16, kind="ExternalInput")
    wq = nc.dram_tensor("wq", [128, KT * 1024], BF16, kind="ExternalInput")
    wk = nc.dram_tensor("wk", [128, KT * 1024], F8, kind="ExternalInput")
    wv = nc.dram_tensor("wv", [128, KT * 1024], BF16, kind="ExternalInput")
    wo = nc.dram_tensor("wo", [128, KT * 1024], BF16, kind="ExternalInput")
    qmask = nc.dram_tensor("qmask", [128, TT * 128], BF16, kind="ExternalInput")
    ones_bf = nc.dram_tensor("ones_bf", [128, 64], BF16, kind="ExternalInput")
    ones_fr = nc.dram_tensor("ones_fr", [128, 64], F32R, kind="ExternalInput")
    out = nc.dram_tensor("out", [ROWS_PER_CORE, DIM], F32, kind="ExternalOutput")

    n_pairs = {"v": 0, "vk": 8, "vka": 8, "all": 8}[phases]
    do_v = True
    do_attn = phases in ("vka", "all")
    do_proj = phases == "all"

    with tile.TileContext(nc) as tc:
        with tc.tile_pool(name="persist", bufs=1) as pers, \
             tc.tile_pool(name="work", bufs=3) as work, \
             tc.tile_pool(name="epool", bufs=9) as epool, \
             tc.tile_pool(name="wstream", bufs=2) as wstream, \
             tc.tile_pool(name="pavsb", bufs=8) as pavsb, \
             tc.tile_pool(name="psmm", bufs=2, space="PSUM") as psmm, \
             tc.tile_pool(name="psst", bufs=2, space="PSUM") as psst, \
             tc.tile_pool(name="psav", bufs=2, space="PSUM") as psav:

            ones_sb = pers.tile([128, 64], F32R, tag="ones")
            onesb_sb = pers.tile([128, 64], BF16, tag="onesb")
            qm_sb = pers.tile([128, TT * 128], BF16, tag="qmask")
            xT4 = xT[:].rearrange("p (k t c) -> p k t c", k=KT, t=TT, c=128)
            xp8_sb = pers.tile([128, KT * T], F8, tag="xp8")
            # [p, jpair, e, t] — e selects kt=2j+e (DoubleRow pair)
            xp3 = xp8_sb[:].rearrange("p (j e t) -> p j e t", j=KT // 2, e=2)
            xq_sb = pers.tile([128, KT * 512], BF16, tag="xq")
            v_sb = pers.tile([128, TT * 1040], BF16, tag="v")
            v4 = v_sb[:].rearrange("p (t h e) -> p t h e", t=TT, h=N_HEADS, e=65)
            aot_sb = [
                pers.tile([128, 512], BF16, tag=f"aot{p}", name=f"aot{p}")
                for p in range(8)
            ]
            wo_sb = pers.tile([128, KT * DIM], BF16, tag="wo")

            for _rep in range(reps):
                nc.sync.dma_start(out=ones_sb[:], in_=ones_fr[:])
                nc.sync.dma_start(out=onesb_sb[:], in_=ones_bf[:])
                nc.sync.dma_start(out=qm_sb[:], in_=qmask[:])
                for h in range(N_HEADS):
                    nc.vector.tensor_copy(
                        out=v4[:, :, h, 64:65],
                        in_=onesb_sb[:, 0:TT].rearrange("p (t o) -> p t o", o=1),
                    )
                if not do_attn:
                    for p_ in range(8):
                        nc.vector.tensor_copy(out=aot_sb[p_][:, 0:64], in_=onesb_sb[:])

                # ---- V pass, pipelined with tt-major x streaming ----
                # x (bf16) is only read by the V pass; scope it so its SBUF
                # is reclaimed for the attention working set
                with tc.tile_pool(name=f"wvpool{_rep}", bufs=1) as wvpool:
                    x_sb = wvpool.tile(
                        [128, KT * T], BF16, tag="x", name=f"x_{_rep}"
                    )
                    x4 = x_sb[:].rearrange(
                        "p (k t c) -> p k t c", k=KT, t=TT, c=128
                    )
                    wv_sb = wvpool.tile(
                        [128, KT * 1024], BF16, tag="wv", name=f"wv_{_rep}"
                    )
                    # order: wv half0, first x tiles, wv half1, rest of x —
                    # V math starts after ~1.25MB of DMA instead of ~2.25MB
                    nc.sync.dma_start(out=wv_sb[:, 0:4096], in_=wv[:, 0:4096])
                    # x arrives token-tile-major: all 8 kt slices of tile tt
                    # in one strided DMA, so V math starts after tile 0
                    for tt in range(2):
                        nc.sync.dma_start(out=x4[:, :, tt, :], in_=xT4[:, :, tt, :])
                    nc.sync.dma_start(
                        out=wv_sb[:, 4096:8192], in_=wv[:, 4096:8192]
                    )
                    for tt in range(2, TT):
                        nc.sync.dma_start(out=x4[:, :, tt, :], in_=xT4[:, :, tt, :])
                    nc.sync.dma_start(out=xp8_sb[:], in_=xp8[:])
                    nc.sync.dma_start(out=xq_sb[:], in_=xq[:])
                    if do_proj:
                        # prefetch: proj weights land during attention
                        for q2 in range(2):
                            nc.sync.dma_start(
                                out=wo_sb[:, q2 * 4096 : (q2 + 1) * 4096],
                                in_=wo[:, q2 * 4096 : (q2 + 1) * 4096],
                            )
                    if do_v:
                        for tt in range(TT):
                            for half in range(2):
                                vacc = psmm.tile([128, 512], F32, tag="mm512")
                                for kt in range(KT):
                                    nc.tensor.matmul(
                                        vacc[:],
                                        x4[:, kt, tt, :],
                                        wv_sb[:, half * 4096 + kt * 512 : half * 4096 + kt * 512 + 512],
                                        start=(kt == 0), stop=(kt == KT - 1),
                                    )
                                nc.vector.tensor_copy(
                                    out=v4[:, tt, half * 8 : half * 8 + 8, 0:64],
                                    in_=vacc[:].rearrange("p (h e) -> p h e", h=8, e=64),
                                )

                # ---- per head-pair: K.T, Q.T (fp8 DoubleRow), attention ----
                pav_drained = []
                for p in range(n_pairs):
                    wk_p = wstream.tile([128, KT * 128], F8, tag="wkp")
                    wq_p = wstream.tile([128, KT * 128], BF16, tag="wqp")
                    nc.sync.dma_start(
                        out=wk_p[:], in_=wk[:, p * 1024 : p * 1024 + 1024]
                    )
                    nc.sync.dma_start(
                        out=wq_p[:], in_=wq[:, p * 1024 : p * 1024 + 1024]
                    )
                    wk3 = wk_p[:].rearrange("p (j e d) -> p j e d", j=KT // 2, e=2)

                    ktp = wstream.tile([128, T], BF16, tag="ktp")
                    for tch in range(4):
                        kacc = psmm.tile([128, 512], F32, tag="mm512")
                        for j in range(KT // 2):
                            nc.tensor.matmul(
                                kacc[:],
                                wk3[:, j],
                                xp3[:, j, :, tch * 512 : tch * 512 + 512],
                                start=(j == 0), stop=(j == KT // 2 - 1),
                                perf_mode=mybir.MatmulPerfMode.DoubleRow,
                            )
                        nc.vector.tensor_copy(
                            out=ktp[:, tch * 512 : tch * 512 + 512], in_=kacc[:]
                        )

                    qtp = wstream.tile([128, 512], BF16, tag="qtp")
                    qacc = psmm.tile([128, 512], F32, tag="mm512")
                    for kt in range(KT):
                        nc.tensor.matmul(
                            qacc[:],
                            wq_p[:, kt * 128 : kt * 128 + 128],
                            xq_sb[:, kt * 512 : kt * 512 + 512],
                            start=(kt == 0), stop=(kt == KT - 1),
                        )
                    nc.vector.tensor_copy(out=qtp[:], in_=qacc[:])

                    if do_attn:
                        pavs = [psav.tile([65, 512], F32, tag="pav", name=f"pav{p}_{half_}_{_rep}") for half_ in range(2)]
                        pending = []  # AV issue lag-2 behind S/exp
                        for g in range(8):
                            j = g // 2
                            qs = 128 * j
                            N = 512 - qs
                            e2s = []
                            for half in range(2):
                                lo, hi = half * 64, half * 64 + 64
                                e2 = epool.tile([128, 1024], BF16, tag="e2")
                                e3 = e2[:].rearrange("p (s m) -> p s m", s=2)
                                st2 = psst.tile([128, 1024], F32, tag="st")
                                st3 = st2[:].rearrange("p (s m) -> p s m", s=2)
                                for s in range(2):
                                    kt = 2 * g + s
                                    nc.tensor.matmul(
                                        st2[:, s * 512 : s * 512 + N],
                                        ktp[lo:hi, kt * 128 : kt * 128 + 128],
                                        qtp[lo:hi, qs:512],
                                        start=True, stop=True,
                                    )
                                if probe == "noexp":
                                    nc.vector.tensor_copy(
                                        out=e3[:, :, 0:N], in_=st3[:, :, 0:N]
                                    )
                                else:
                                    nc.scalar.activation(
                                        e3[:, :, 0:N], st3[:, :, 0:N],
                                        mybir.ActivationFunctionType.Exp,
                                        scale=SCORE_SCALE,
                                    )
                                # diag-mask multiply on the otherwise-idle
                                # gpsimd engine; DVE keeps the PSUM drains
                                nc.gpsimd.tensor_mul(
                                    out=e3[:, :, 0:128],
                                    in0=e3[:, :, 0:128],
                                    in1=qm_sb[:, 2 * g * 128 : 2 * g * 128 + 256].rearrange(
                                        "p (s m) -> p s m", s=2
                                    ),
                                )
                                e2s.append(e2)
                            pending.append((e2s, j, N, g))
                            if len(pending) > 3:
                                _issue_av2(nc, pavs, v_sb, pending.pop(0), p)
                        for pend in pending:
                            _issue_av2(nc, pavs, v_sb, pend, p)

                        # drain pav to SBUF; normalization happens after the
                        # pair loop so its recip->matmul chains never stall
                        # the hot PE stream
                        pv = pavsb.tile([65, 1024], BF16, tag="pavS",
                                        name=f"pavS{p}_{_rep}")
                        pav_drained.append(pv)
                        for half in range(2):
                            nc.vector.tensor_copy(
                                out=pv[:, half * 512 : half * 512 + 512],
                                in_=pavs[half][:],
                            )

                # ---- deferred softmax normalization ----
                if do_attn:
                    for p in range(n_pairs):
                        pv = pav_drained[p]
                        for half in range(2):
                            lo, hi = half * 64, half * 64 + 64
                            sl = pv[:, half * 512 : half * 512 + 512]
                            recip = work.tile([128, 512], F32R, tag="recip")
                            with nc.allow_low_precision(reason="softmax recip; f32r ~1e-4"):
                                nc.vector.reciprocal(out=recip[64:65, :], in_=sl[64:65, :])
                            pb = psmm.tile([64, 512], F32, tag="mm512")
                            nc.tensor.matmul(
                                pb[:], ones_sb[64:65, 0:64], recip[64:65, :],
                                start=True, stop=True,
                            )
                            nc.vector.tensor_mul(
                                out=aot_sb[p][lo:hi, :], in0=sl[0:64, :], in1=pb[:]
                            )

                # ---- projection ----
                if do_proj:
                    for qi in range(4):
                        for fc in range(2):
                            yp = psmm.tile([128, 512], F32, tag="mm512")
                            for p in range(8):
                                nc.tensor.matmul(
                                    yp[:],
                                    aot_sb[p][:, qi * 128 : qi * 128 + 128],
                                    wo_sb[:, p * DIM + fc * 512 : p * DIM + fc * 512 + 512],
                                    start=(p == 0), stop=(p == 7),
                                )
                            y_sb = work.tile([128, 512], F32, tag="y")
                            nc.vector.tensor_copy(out=y_sb[:], in_=yp[:])
                            nc.sync.dma_start(
                                out=out[qi * 128 : qi * 128 + 128, fc * 512 : fc * 512 + 512],
                                in_=y_sb[:],
                            )

    nc.finalize()
    return nc


def _issue_av2(nc, pavs, v_sb, pend, p):
    e2s, j, N, g = pend
    qs = 128 * j
    for half in range(2):
        h = 2 * p + half
        for s in range(2):
            kt = 2 * g + s
            nc.tensor.matmul(
                pavs[half][:, qs:512],
                v_sb[:, kt * 1040 + h * 65 : kt * 1040 + h * 65 + 65],
                e2s[half][:, s * 512 : s * 512 + N],
                start=(kt == 0), stop=(kt == TT - 1),
            )


def make_in_maps(x, W_qkv, W_proj):
    import ml_dtypes

    bf = ml_dtypes.bfloat16
    f8 = mybir.dt.np(mybir.dt.float8e4)
    x = np.asarray(x, dtype=np.float32)
    W_qkv = np.asarray(W_qkv, dtype=np.float32)
    W_proj = np.asarray(W_proj, dtype=np.float32)
    W_q, W_k, W_v = W_qkv[:DIM], W_qkv[DIM:2 * DIM], W_qkv[2 * DIM:]

    def pair_major_f8(W):
        # [p, head-pair, jpair, e, d] with kt = 2j+e (DoubleRow interleave);
        # prescaled so fp8 encodings avoid the subnormal range
        WT = (W.T * WQK_PRESCALE).reshape(KT // 2, 2, 128, 8, 128)
        return np.ascontiguousarray(
            WT.transpose(2, 3, 0, 1, 4).reshape(128, KT * 1024)
        ).astype(f8)

    def pair_major(W):
        WT = W.T.reshape(KT, 128, 8, 128)
        return np.ascontiguousarray(
            WT.transpose(1, 2, 0, 3).reshape(128, KT * 1024)
        ).astype(bf)

    wq_d = pair_major(W_q)
    wk_d = pair_major_f8(W_k)
    wv_d = np.ascontiguousarray(
        W_v.T.reshape(KT, 128, 2, 512).transpose(1, 2, 0, 3).reshape(128, KT * 1024)
    ).astype(bf)
    wo_d = np.ascontiguousarray(
        W_proj.T.reshape(KT, 128, DIM).transpose(1, 0, 2).reshape(128, KT * DIM)
    ).astype(bf)

    in_maps = []
    for core in range(N_CORES):
        b, c = core // 4, core % 4
        xb = x[b]
        xTl = np.ascontiguousarray(
            xb.T.reshape(KT, 128, T).transpose(1, 0, 2).reshape(128, KT * T)
        ).astype(bf)
        # fp8 kt-pair-interleaved copy for the DoubleRow K projection
        xp8l = np.ascontiguousarray(
            xb.T.reshape(KT // 2, 2, 128, T).transpose(2, 0, 1, 3).reshape(128, KT * T)
        ).astype(f8)
        qrows = np.arange(512) * 4 + c  # gathered rows, ascending
        xql = np.ascontiguousarray(
            xb[qrows].T.reshape(KT, 128, 512).transpose(1, 0, 2).reshape(128, KT * 512)
        ).astype(bf)
        # diag masks: for k-tile kt, q-tile j=kt//4 occupies active cols [0:128]:
        # qmask[:, kt*128 + i] = 1 if kt*128 + p <= qrows[128*(kt//4) + i]
        p_idx = np.arange(128)[:, None]
        qm = np.empty((128, TT * 128), dtype=bf)
        for kt in range(TT):
            jj = kt // 4
            qr = qrows[128 * jj : 128 * jj + 128][None, :]
            qm[:, kt * 128 : kt * 128 + 128] = (kt * 128 + p_idx <= qr).astype(bf)
        in_maps.append({
            "xT": xTl, "xp8": xp8l, "xq": xql,
            "wq": wq_d, "wk": wk_d, "wv": wv_d, "wo": wo_d,
            "qmask": qm,
            "ones_bf": np.ones((128, 64), bf),
            "ones_fr": np.ones((128, 64), np.float32),
        })
    return in_maps


def assemble_output(results):
    y = np.empty((B, T, DIM), dtype=np.float32)
    for core in range(N_CORES):
        b, c = core // 4, core % 4
        y[b, np.arange(512) * 4 + c] = results[core]["out"]
    return y


def kernel(x, W_qkv, W_proj):
    from concourse.bass_utils import run_bass_kernel_spmd

    if "nc" not in _CACHE:
        _CACHE["nc"] = build_nc()
    nc = _CACHE["nc"]
    in_maps = make_in_maps(x, W_qkv, W_proj)
    res = run_bass_kernel_spmd(nc, in_maps, list(range(N_CORES)))
    return assemble_output(res.results)
